# revision 10
# baseline (speedup 1.0000x reference)
"""TRN2 Bass/Tile kernel: 16-head causal multi-head attention.

Problem: x[2,2048,1024], 16 heads x 64, causal softmax attention + out-proj.

Sharding (8 cores): core = b*4 + g  (b = batch 0..1, g = head-group 0..3).
Each core computes heads [4g, 4g+4) for batch b and the partial
out-projection  ctx_g @ Wo[g*256:(g+1)*256, :]  -> [2048, 1024].
Host sums the 4 partials per batch and adds bo.

On-device layout is fully "transposed" (feature-major):
  xT   [128, 8, 2048]  : xT[p, kt, s]  = x[b, s, kt*128+p]
  QT/KT[128, 2, 2048]  : QT[p, t, s]   = Q^T[t*128+p, s]   (d' = h*64+j on partitions)
  S^T  [128k, 512q]    : per (head, q-chunk, k-tile) block = K @ Q^T
  softmax: no max-subtraction (scores are O(1) by construction: exp is safe);
  denominators via a ones-column appended to V (row 64 of the ctx psum);
  ctx^T [128, 2, 2048] feeds the out-projection directly as lhsT.

Schedule notes (v2): chunk-outer attention; all S^T/exp for a chunk are
emitted before the ctx matmuls so the in-order PE queue is never blocked
behind an exp dependency (that ping-pong caused a 2x cadence and HAM
re-throttle in v1); out-projection runs per chunk; PSUM evacuations go to
DVE, keeping ScalarE exp-only (ACT is the critical engine).
"""

import os
import sys

for _p in ("/opt/trn_rl_repo",):
    if _p not in sys.path:
        sys.path.insert(0, _p)

import numpy as np

import concourse.bass as bass
import concourse.mybir as mybir
import concourse.tile as tile
from concourse import bacc
from concourse.bass import ts
from concourse.bass_utils import run_bass_kernel_spmd

B, S, D, H, HD = 2, 2048, 1024, 16, 64
GROUPS, HPG, DG = 4, 4, 256  # head groups, heads/group, group width
KT = D // 128  # 8 k-tiles over D
ST = S // 128  # 16 s-tiles
CH = 512  # q-chunk width
QCH = S // CH  # 4 q-chunks
F32 = mybir.dt.float32

# matmul input dtype knob
_MM_DT_NAME = os.environ.get("BASS_MM_DT", "f32r")
MM_DT = {
    "f32r": mybir.dt.float32r,
    "f32": mybir.dt.float32,
    "bf16": mybir.dt.bfloat16,
}[_MM_DT_NAME]
RECIP_MODE = os.environ.get("BASS_RECIP", "plain")  # plain | approx


def _np_dt():
    import ml_dtypes

    return ml_dtypes.bfloat16 if _MM_DT_NAME == "bf16" else np.float32


def build_kernel_body(nc, tc, io):
    Exp = mybir.ActivationFunctionType.Exp

    consts = tc.alloc_tile_pool(name="consts", bufs=1)
    acts = tc.alloc_tile_pool(name="acts", bufs=1)
    xtp = tc.alloc_tile_pool(name="xtp", bufs=1)

    # ---- constant loads -------------------------------------------------
    wq_sb = consts.tile([128, KT, DG], MM_DT)
    nc.sync.dma_start(out=wq_sb, in_=io["wq"])
    wk_sb = consts.tile([128, KT, DG], MM_DT)
    nc.sync.dma_start(out=wk_sb, in_=io["wk"])
    wv_sb = consts.tile([128, KT, DG], MM_DT)
    nc.sync.dma_start(out=wv_sb, in_=io["wv"])
    wo_sb = consts.tile([128, 2, 1024], MM_DT)
    nc.sync.dma_start(out=wo_sb, in_=io["wo"])
    bq_sb = consts.tile([128, 2], F32)
    nc.sync.dma_start(out=bq_sb, in_=io["bq"])
    bk_sb = consts.tile([128, 2], F32)
    nc.sync.dma_start(out=bk_sb, in_=io["bk"])
    vb_sb = consts.tile([128, HPG, HD], F32)
    nc.sync.dma_start(out=vb_sb, in_=io["vb"])
    ones_sb = consts.tile([1, HD], MM_DT)
    nc.sync.dma_start(out=ones_sb, in_=io["onesd"])

    xt_sb = xtp.tile([128, KT, S], MM_DT)
    for kt in range(KT):
        nc.sync.dma_start(out=xt_sb[:, kt, :], in_=io["xt"][:, kt, :])

    # ---- persistent activations ----------------------------------------
    qt_sb = acts.tile([128, 2, S], MM_DT)  # Q^T (pre-scaled by 1/8 via host W/b)
    kt_sb = acts.tile([128, 2, S], MM_DT)  # K^T
    v_sb = acts.tile([128, ST, HPG, HD + 1], MM_DT)  # V blocks + ones column
    ctxT_sb = acts.tile([128, 2, S], MM_DT)  # normalized ctx^T

    # ---- projections (own PSUM pool) ------------------------------------
    pps = tc.alloc_tile_pool(name="pps", bufs=3, space="PSUM")
    # V natural: V[st*128+p, h*64+j] ; ones column at j=64 (DMA'd constant)
    nc.sync.dma_start(out=v_sb[:, :, :, HD : HD + 1], in_=io["vones"])
    for st in range(ST):
        v_ps = pps.tile([128, DG], F32, tag="mm")
        for kt in range(KT):
            nc.tensor.matmul(
                v_ps,
                lhsT=xt_sb[:, kt, ts(st, 128)],
                rhs=wv_sb[:, kt, :],
                start=(kt == 0),
                stop=(kt == KT - 1),
            )
        nc.vector.tensor_add(
            out=v_sb[:, st, :, 0:HD],
            in0=v_ps.rearrange("p (h j) -> p h j", h=HPG),
            in1=vb_sb,
        )

    # Q^T / K^T: lhsT = W (natural), rhs = x^T ; bias-add on DVE
    for t in range(2):
        for c in range(QCH):
            q_ps = pps.tile([128, CH], F32, tag="mm")
            for kt in range(KT):
                nc.tensor.matmul(
                    q_ps,
                    lhsT=wq_sb[:, kt, ts(t, 128)],
                    rhs=xt_sb[:, kt, ts(c, CH)],
                    start=(kt == 0),
                    stop=(kt == KT - 1),
                )
            nc.vector.tensor_scalar_add(
                out=qt_sb[:, t, ts(c, CH)], in0=q_ps, scalar1=bq_sb[:, t : t + 1]
            )
            k_ps = pps.tile([128, CH], F32, tag="mm")
            for kt in range(KT):
                nc.tensor.matmul(
                    k_ps,
                    lhsT=wk_sb[:, kt, ts(t, 128)],
                    rhs=xt_sb[:, kt, ts(c, CH)],
                    start=(kt == 0),
                    stop=(kt == KT - 1),
                )
            nc.vector.tensor_scalar_add(
                out=kt_sb[:, t, ts(c, CH)], in0=k_ps, scalar1=bk_sb[:, t : t + 1]
            )

    xtp.release()
    pps.release()

    work = tc.alloc_tile_pool(name="work", bufs=8)
    small = tc.alloc_tile_pool(name="small", bufs=2)
    apses = tc.alloc_tile_pool(name="apses", bufs=2, space="PSUM")

    # ---- attention + out-projection, chunk-outer -------------------------
    PIPE = 2  # ctx matmuls trail the S^T/exp stream by this many k-steps
    for c in range(QCH):
        nkt = (c + 1) * (CH // 128)  # causal: k-tiles 0..nkt-1
        ctx_ps = [
            apses.tile([HD + 1, CH], F32, tag="ctx", bufs=4, name=f"ctx_ps{_h}")
            for _h in range(HPG)
        ]
        exp_sb = [[None] * nkt for _ in range(HPG)]

        def emit_scores(i, c=c, exp_sb=exp_sb):
            off = max(0, 128 * i - CH * c)  # first unmasked column in chunk
            for h in range(HPG):
                t, pb = h // 2, (h % 2) * 64
                sT_ps = apses.tile([128, CH], F32, tag="sT", bufs=2)
                nc.tensor.matmul(
                    sT_ps[:, off:CH],
                    lhsT=kt_sb[pb : pb + HD, t, ts(i, 128)],
                    rhs=qt_sb[pb : pb + HD, t, c * CH + off : (c + 1) * CH],
                    start=True,
                    stop=True,
                )
                e = work.tile([128, CH], MM_DT, tag="exp", bufs=16)
                nc.scalar.activation(out=e[:, off:CH], in_=sT_ps[:, off:CH], func=Exp)
                if 128 * i + 128 > CH * c + off:  # crosses the diagonal: mask
                    nc.gpsimd.affine_select(
                        out=e[:, off:CH],
                        in_=e[:, off:CH],
                        pattern=[[1, CH - off]],
                        base=off - (128 * i - CH * c),
                        channel_multiplier=-1,
                        compare_op=mybir.AluOpType.is_ge,
                        fill=0.0,
                    )
                exp_sb[h][i] = (e, off)

        def emit_ctx(i, nkt=nkt, ctx_ps=ctx_ps, exp_sb=exp_sb):
            for h in range(HPG):
                e, off = exp_sb[h][i]
                nc.tensor.matmul(
                    ctx_ps[h][:, off:CH],
                    lhsT=v_sb[:, i, h, :],
                    rhs=e[:, off:CH],
                    start=(i == 0),
                    stop=(i == nkt - 1),
                )

        for i in range(nkt + PIPE):
            if i < nkt:
                emit_scores(i)
            if i >= PIPE:
                emit_ctx(i - PIPE)
        # phase 3: normalize -> ctxT
        for h in range(HPG):
            t, pb = h // 2, (h % 2) * 64
            recip_mm = small.tile([1, CH], MM_DT, tag="recip")
            if RECIP_MODE == "approx":
                rf32 = small.tile([1, CH], F32, tag="rf32")
                rscr = small.tile([1, CH], F32, tag="rscr")
                nc.vector.reciprocal_approx_accurate(
                    out=rf32, in_=ctx_ps[h][HD : HD + 1, :], scratch=rscr
                )
                nc.vector.tensor_copy(out=recip_mm, in_=rf32)
            else:
                nc.vector.reciprocal(out=recip_mm, in_=ctx_ps[h][HD : HD + 1, :])
            bc_ps = apses.tile([HD, CH], F32, tag="bc", bufs=2)
            nc.tensor.matmul(bc_ps, lhsT=ones_sb, rhs=recip_mm, start=True, stop=True)
            bc_sb = small.tile([HD, CH], F32, tag="bc_sb")
            nc.vector.tensor_copy(out=bc_sb, in_=bc_ps)
            if pb == 0:
                nc.vector.tensor_mul(
                    out=ctxT_sb[0:HD, t, ts(c, CH)], in0=ctx_ps[h][0:HD, :], in1=bc_sb
                )
            else:
                stg_sb = small.tile([HD, CH], MM_DT, tag="stg")
                nc.vector.tensor_mul(out=stg_sb, in0=ctx_ps[h][0:HD, :], in1=bc_sb)
                # DVE cannot shift partitions; bounce via SBUF->SBUF DMA
                nc.sync.dma_start(out=ctxT_sb[pb : pb + HD, t, ts(c, CH)], in_=stg_sb)
        # phase 4: out-projection for this chunk's 4 s-tiles
        for st in range(4 * c, 4 * c + 4):
            o_sb = work.tile([128, 1024], F32, tag="osb", bufs=3)
            for nch in range(2):
                o_ps = apses.tile([128, CH], F32, tag="bc", bufs=2)
                for t in range(2):
                    nc.tensor.matmul(
                        o_ps,
                        lhsT=ctxT_sb[:, t, ts(st, 128)],
                        rhs=wo_sb[:, t, ts(nch, CH)],
                        start=(t == 0),
                        stop=(t == 1),
                    )
                nc.vector.tensor_copy(out=o_sb[:, ts(nch, CH)], in_=o_ps)
            nc.sync.dma_start(out=io["out"][ts(st, 128), :], in_=o_sb)

    apses.release()
    small.release()
    work.release()
    acts.release()
    consts.release()


def build_nc():
    nc = bacc.Bacc("TRN2", target_bir_lowering=False, debug=False)
    io = {
        "xt": nc.dram_tensor("xt", [128, KT, S], MM_DT, kind="ExternalInput").ap(),
        "wq": nc.dram_tensor("wq", [128, KT, DG], MM_DT, kind="ExternalInput").ap(),
        "wk": nc.dram_tensor("wk", [128, KT, DG], MM_DT, kind="ExternalInput").ap(),
        "wv": nc.dram_tensor("wv", [128, KT, DG], MM_DT, kind="ExternalInput").ap(),
        "wo": nc.dram_tensor("wo", [128, 2, 1024], MM_DT, kind="ExternalInput").ap(),
        "bq": nc.dram_tensor("bq", [128, 2], F32, kind="ExternalInput").ap(),
        "bk": nc.dram_tensor("bk", [128, 2], F32, kind="ExternalInput").ap(),
        "vb": nc.dram_tensor("vb", [128, HPG, HD], F32, kind="ExternalInput").ap(),
        "onesd": nc.dram_tensor("onesd", [1, HD], MM_DT, kind="ExternalInput").ap(),
        "vones": nc.dram_tensor(
            "vones", [128, ST, HPG, 1], MM_DT, kind="ExternalInput"
        ).ap(),
        "out": nc.dram_tensor("out", [S, D], F32, kind="ExternalOutput").ap(),
    }
    with tile.TileContext(nc) as tc, nc.allow_low_precision(
        reason="reduced-precision matmul operand pipeline; accumulation stays fp32"
    ):
        build_kernel_body(nc, tc, io)
    nc.compile()
    return nc


_NC = None


def get_nc():
    global _NC
    if _NC is None:
        _NC = build_nc()
    return _NC


def _tile_rows(a, p=128):
    """[R, N] -> [128, R//128, N] with row r = kt*128 + p."""
    r, n = a.shape
    return np.ascontiguousarray(a.reshape(r // p, p, n).transpose(1, 0, 2)).astype(
        _np_dt()
    )


def shard_inputs(x, Wq, bq, Wk, bk, Wv, bv, Wo, bo):
    scale = 1.0 / np.sqrt(np.float32(HD))
    in_maps = []
    for core in range(8):
        b, g = divmod(core, GROUPS)
        sl = slice(g * DG, (g + 1) * DG)
        vb = np.ascontiguousarray(
            np.broadcast_to(bv[sl].reshape(HPG, HD)[None], (128, HPG, HD))
        ).astype(np.float32)
        in_maps.append(
            {
                "xt": _tile_rows(np.ascontiguousarray(x[b].T)),
                "wq": _tile_rows(np.ascontiguousarray(Wq[:, sl]) * scale),
                "wk": _tile_rows(np.ascontiguousarray(Wk[:, sl])),
                "wv": _tile_rows(np.ascontiguousarray(Wv[:, sl])),
                "wo": _tile_rows(np.ascontiguousarray(Wo[sl, :])),
                "bq": np.ascontiguousarray((bq[sl] * scale).reshape(2, 128).T),
                "bk": np.ascontiguousarray(bk[sl].reshape(2, 128).T),
                "vb": vb,
                "onesd": np.ones((1, HD), _np_dt()),
                "vones": np.ones((128, ST, HPG, 1), _np_dt()),
            }
        )
    return in_maps


LAST_RESULT = None


def kernel(**inputs):
    global LAST_RESULT
    nc = get_nc()
    in_maps = shard_inputs(**inputs)
    trace = bool(int(os.environ.get("BASS_KERNEL_TRACE", "0")))
    res = run_bass_kernel_spmd(nc, in_maps, core_ids=list(range(8)), trace=trace)
    LAST_RESULT = res
    parts = [res.results[c]["out"] for c in range(8)]
    out = np.stack(
        [
            parts[0] + parts[1] + parts[2] + parts[3],
            parts[4] + parts[5] + parts[6] + parts[7],
        ]
    )
    return (out + inputs["bo"]).astype(np.float32)


# revision 11
# speedup vs baseline: 1.0152x; 1.0152x over previous
"""TRN2 Bass/Tile kernel: 16-head causal multi-head attention.

Problem: x[2,2048,1024], 16 heads x 64, causal softmax attention + out-proj.

Sharding (8 cores): core = b*4 + g  (b = batch 0..1, g = head-group 0..3).
Each core computes heads [4g, 4g+4) for batch b and the partial
out-projection  ctx_g @ Wo[g*256:(g+1)*256, :]  -> [2048, 1024].
Host sums the 4 partials per batch and adds bo.

On-device layout is fully "transposed" (feature-major):
  xT   [128, 8, 2048]  : xT[p, kt, s]  = x[b, s, kt*128+p]
  QT/KT[128, 2, 2048]  : QT[p, t, s]   = Q^T[t*128+p, s]   (d' = h*64+j on partitions)
  S^T  [128k, 512q]    : per (head, q-chunk, k-tile) block = K @ Q^T
  softmax: no max-subtraction (scores are O(1) by construction: exp is safe);
  denominators via a ones-column appended to V (row 64 of the ctx psum);
  ctx^T [128, 2, 2048] feeds the out-projection directly as lhsT.

Schedule (v3):
  proj(0) proj(1) attn(0) proj(2) tail(0) attn(1) proj(3) tail(1)
  attn(2) tail(2) attn(3) tail(3)
where proj(c) = Q^T/K^T chunk c + V s-tiles of chunk c, attn(c) = the
S^T/exp stream with ctx matmuls trailing by PIPE k-steps, tail(c) =
softmax normalization + out-projection of chunk c.  Tails are emitted
one chunk late so their serial DVE reciprocals and the bc/out-proj
matmuls hide behind the next chunk's dense PE work (v2 stalled the
in-order PE queue ~13us per chunk boundary on reciprocals, which also
re-throttled the PE clock via HAM).  ScalarE runs exp only; all PSUM
evacuation is on DVE.
"""

import os
import sys

for _p in ("/opt/trn_rl_repo",):
    if _p not in sys.path:
        sys.path.insert(0, _p)

import numpy as np

import concourse.bass as bass
import concourse.mybir as mybir
import concourse.tile as tile
from concourse import bacc
from concourse.bass import ts
from concourse.bass_utils import run_bass_kernel_spmd

B, S, D, H, HD = 2, 2048, 1024, 16, 64
GROUPS, HPG, DG = 4, 4, 256  # head groups, heads/group, group width
KT = D // 128  # 8 k-tiles over D
ST = S // 128  # 16 s-tiles
CH = 512  # q-chunk width
QCH = S // CH  # 4 q-chunks
PIPE = 2  # ctx matmuls trail the S^T/exp stream by this many k-steps
F32 = mybir.dt.float32

# matmul input dtype knob
_MM_DT_NAME = os.environ.get("BASS_MM_DT", "bf16")
MM_DT = {
    "f32r": mybir.dt.float32r,
    "f32": mybir.dt.float32,
    "bf16": mybir.dt.bfloat16,
}[_MM_DT_NAME]


def _np_dt():
    import ml_dtypes

    return ml_dtypes.bfloat16 if _MM_DT_NAME == "bf16" else np.float32


def build_kernel_body(nc, tc, io):
    Exp = mybir.ActivationFunctionType.Exp

    consts = tc.alloc_tile_pool(name="consts", bufs=1)
    acts = tc.alloc_tile_pool(name="acts", bufs=1)
    work = tc.alloc_tile_pool(name="work", bufs=2)
    small = tc.alloc_tile_pool(name="small", bufs=2)
    psum = tc.alloc_tile_pool(name="psum", bufs=1, space="PSUM")

    # ---- constant loads -------------------------------------------------
    wq_sb = consts.tile([128, KT, DG], MM_DT)
    nc.sync.dma_start(out=wq_sb, in_=io["wq"])
    wk_sb = consts.tile([128, KT, DG], MM_DT)
    nc.sync.dma_start(out=wk_sb, in_=io["wk"])
    wv_sb = consts.tile([128, KT, DG], MM_DT)
    nc.sync.dma_start(out=wv_sb, in_=io["wv"])
    wo_sb = consts.tile([128, 2, 1024], MM_DT)
    nc.sync.dma_start(out=wo_sb, in_=io["wo"])
    bq_sb = consts.tile([128, 2], F32)
    nc.sync.dma_start(out=bq_sb, in_=io["bq"])
    bk_sb = consts.tile([128, 2], F32)
    nc.sync.dma_start(out=bk_sb, in_=io["bk"])
    vb_sb = consts.tile([128, HPG, HD], F32)
    nc.sync.dma_start(out=vb_sb, in_=io["vb"])
    ones_sb = consts.tile([1, HD], MM_DT)
    nc.sync.dma_start(out=ones_sb, in_=io["onesd"])

    xt_sb = consts.tile([128, KT, S], MM_DT)
    for kt in range(KT):
        nc.sync.dma_start(out=xt_sb[:, kt, :], in_=io["xt"][:, kt, :])

    # ---- persistent activations ----------------------------------------
    qt_sb = acts.tile([128, 2, S], MM_DT)  # Q^T (pre-scaled by 1/8 via host W/b)
    kt_sb = acts.tile([128, 2, S], MM_DT)  # K^T
    v_sb = acts.tile([128, ST, HPG, HD + 1], MM_DT)  # V blocks + ones column
    ctxT_sb = acts.tile([128, 2, S], MM_DT)  # normalized ctx^T
    nc.sync.dma_start(out=v_sb[:, :, :, HD : HD + 1], in_=io["vones"])

    def emit_proj(c):
        """Q^T/K^T for chunk c and V s-tiles 4c..4c+3 (PE filler work)."""
        for t in range(2):
            q_ps = psum.tile([128, CH], F32, tag="mm", name=f"q_ps{c}{t}")
            for kt in range(KT):
                nc.tensor.matmul(
                    q_ps,
                    lhsT=wq_sb[:, kt, ts(t, 128)],
                    rhs=xt_sb[:, kt, ts(c, CH)],
                    start=(kt == 0),
                    stop=(kt == KT - 1),
                )
            nc.vector.tensor_scalar_add(
                out=qt_sb[:, t, ts(c, CH)], in0=q_ps, scalar1=bq_sb[:, t : t + 1]
            )
            k_ps = psum.tile([128, CH], F32, tag="mm", name=f"k_ps{c}{t}")
            for kt in range(KT):
                nc.tensor.matmul(
                    k_ps,
                    lhsT=wk_sb[:, kt, ts(t, 128)],
                    rhs=xt_sb[:, kt, ts(c, CH)],
                    start=(kt == 0),
                    stop=(kt == KT - 1),
                )
            nc.vector.tensor_scalar_add(
                out=kt_sb[:, t, ts(c, CH)], in0=k_ps, scalar1=bk_sb[:, t : t + 1]
            )
        for st in range(4 * c, 4 * c + 4):
            v_ps = psum.tile([128, DG], F32, tag="mm", name=f"v_ps{st}")
            for kt in range(KT):
                nc.tensor.matmul(
                    v_ps,
                    lhsT=xt_sb[:, kt, ts(st, 128)],
                    rhs=wv_sb[:, kt, :],
                    start=(kt == 0),
                    stop=(kt == KT - 1),
                )
            nc.vector.tensor_add(
                out=v_sb[:, st, :, 0:HD],
                in0=v_ps.rearrange("p (h j) -> p h j", h=HPG),
                in1=vb_sb,
            )

    ctx_of = {}  # c -> list of 4 ctx psum tiles

    def emit_attn(c):
        """S^T/exp stream with ctx matmuls trailing by PIPE k-steps."""
        nkt = (c + 1) * (CH // 128)
        ctx_ps = [
            psum.tile([HD + 1, CH], F32, tag="ctx", bufs=4, name=f"ctx_ps{c}_{h}")
            for h in range(HPG)
        ]
        ctx_of[c] = ctx_ps
        exps = [[None] * nkt for _ in range(HPG)]

        def scores(i):
            off = max(0, 128 * i - CH * c)  # first unmasked column of this k-tile
            for h in range(HPG):
                t, pb = h // 2, (h % 2) * 64
                sT_ps = psum.tile([128, CH], F32, tag="sT", bufs=2, name="sT_ps")
                nc.tensor.matmul(
                    sT_ps[:, off:CH],
                    lhsT=kt_sb[pb : pb + HD, t, ts(i, 128)],
                    rhs=qt_sb[pb : pb + HD, t, c * CH + off : (c + 1) * CH],
                    start=True,
                    stop=True,
                )
                e = work.tile([128, CH], MM_DT, tag="exp", bufs=16, name="e")
                nc.scalar.activation(out=e[:, off:CH], in_=sT_ps[:, off:CH], func=Exp)
                if 128 * i + 128 > CH * c + off:  # crosses the diagonal: mask
                    nc.gpsimd.affine_select(
                        out=e[:, off:CH],
                        in_=e[:, off:CH],
                        pattern=[[1, CH - off]],
                        base=off - (128 * i - CH * c),
                        channel_multiplier=-1,
                        compare_op=mybir.AluOpType.is_ge,
                        fill=0.0,
                    )
                exps[h][i] = (e, off)

        def ctx(i):
            for h in range(HPG):
                e, off = exps[h][i]
                nc.tensor.matmul(
                    ctx_ps[h][:, off:CH],
                    lhsT=v_sb[:, i, h, :],
                    rhs=e[:, off:CH],
                    start=(i == 0),
                    stop=(i == nkt - 1),
                )

        for i in range(nkt + PIPE):
            if i < nkt:
                scores(i)
            if i >= PIPE:
                ctx(i - PIPE)

    def emit_tail(c):
        """Softmax normalization + out-projection for chunk c."""
        ctx_ps = ctx_of.pop(c)
        for h in range(HPG):
            t, pb = h // 2, (h % 2) * 64
            recip_mm = small.tile([1, CH], MM_DT, tag="recip", name="recip_mm")
            nc.vector.reciprocal(out=recip_mm, in_=ctx_ps[h][HD : HD + 1, :])
            bc_ps = psum.tile([HD, CH], F32, tag="bc", bufs=1, name="bc_ps")
            nc.tensor.matmul(bc_ps, lhsT=ones_sb, rhs=recip_mm, start=True, stop=True)
            bc_sb = small.tile([HD, CH], F32, tag="bc_sb", name="bc_sb")
            nc.vector.tensor_copy(out=bc_sb, in_=bc_ps)
            if pb == 0:
                nc.vector.tensor_mul(
                    out=ctxT_sb[0:HD, t, ts(c, CH)], in0=ctx_ps[h][0:HD, :], in1=bc_sb
                )
            else:
                stg_sb = small.tile([HD, CH], MM_DT, tag="stg", name="stg_sb")
                nc.vector.tensor_mul(out=stg_sb, in0=ctx_ps[h][0:HD, :], in1=bc_sb)
                # DVE cannot shift partitions; bounce via SBUF->SBUF DMA
                nc.sync.dma_start(out=ctxT_sb[pb : pb + HD, t, ts(c, CH)], in_=stg_sb)
        for st in range(4 * c, 4 * c + 4):
            o_sb = work.tile([128, 1024], F32, tag="osb", bufs=3, name="o_sb")
            for nch in range(2):
                o_ps = psum.tile([128, CH], F32, tag="bc", bufs=1, name="o_ps")
                for t in range(2):
                    nc.tensor.matmul(
                        o_ps,
                        lhsT=ctxT_sb[:, t, ts(st, 128)],
                        rhs=wo_sb[:, t, ts(nch, CH)],
                        start=(t == 0),
                        stop=(t == 1),
                    )
                nc.vector.tensor_copy(out=o_sb[:, ts(nch, CH)], in_=o_ps)
            nc.sync.dma_start(out=io["out"][ts(st, 128), :], in_=o_sb)

    # ---- pipeline: tails run one chunk late -----------------------------
    emit_proj(0)
    emit_proj(1)
    emit_attn(0)
    emit_proj(2)
    emit_tail(0)
    emit_attn(1)
    emit_proj(3)
    emit_tail(1)
    emit_attn(2)
    emit_tail(2)
    emit_attn(3)
    emit_tail(3)

    psum.release()
    small.release()
    work.release()
    acts.release()
    consts.release()


def build_nc():
    nc = bacc.Bacc("TRN2", target_bir_lowering=False, debug=False)
    io = {
        "xt": nc.dram_tensor("xt", [128, KT, S], MM_DT, kind="ExternalInput").ap(),
        "wq": nc.dram_tensor("wq", [128, KT, DG], MM_DT, kind="ExternalInput").ap(),
        "wk": nc.dram_tensor("wk", [128, KT, DG], MM_DT, kind="ExternalInput").ap(),
        "wv": nc.dram_tensor("wv", [128, KT, DG], MM_DT, kind="ExternalInput").ap(),
        "wo": nc.dram_tensor("wo", [128, 2, 1024], MM_DT, kind="ExternalInput").ap(),
        "bq": nc.dram_tensor("bq", [128, 2], F32, kind="ExternalInput").ap(),
        "bk": nc.dram_tensor("bk", [128, 2], F32, kind="ExternalInput").ap(),
        "vb": nc.dram_tensor("vb", [128, HPG, HD], F32, kind="ExternalInput").ap(),
        "onesd": nc.dram_tensor("onesd", [1, HD], MM_DT, kind="ExternalInput").ap(),
        "vones": nc.dram_tensor(
            "vones", [128, ST, HPG, 1], MM_DT, kind="ExternalInput"
        ).ap(),
        "out": nc.dram_tensor("out", [S, D], F32, kind="ExternalOutput").ap(),
    }
    with tile.TileContext(nc) as tc, nc.allow_low_precision(
        reason="reduced-precision matmul operand pipeline; accumulation stays fp32"
    ):
        build_kernel_body(nc, tc, io)
    nc.compile()
    return nc


_NC = None


def get_nc():
    global _NC
    if _NC is None:
        _NC = build_nc()
    return _NC


def _tile_rows(a, p=128):
    """[R, N] -> [128, R//128, N] with row r = kt*128 + p."""
    r, n = a.shape
    return np.ascontiguousarray(a.reshape(r // p, p, n).transpose(1, 0, 2)).astype(
        _np_dt()
    )


def shard_inputs(x, Wq, bq, Wk, bk, Wv, bv, Wo, bo):
    scale = 1.0 / np.sqrt(np.float32(HD))
    in_maps = []
    for core in range(8):
        b, g = divmod(core, GROUPS)
        sl = slice(g * DG, (g + 1) * DG)
        vb = np.ascontiguousarray(
            np.broadcast_to(bv[sl].reshape(HPG, HD)[None], (128, HPG, HD))
        ).astype(np.float32)
        in_maps.append(
            {
                "xt": _tile_rows(np.ascontiguousarray(x[b].T)),
                "wq": _tile_rows(np.ascontiguousarray(Wq[:, sl]) * scale),
                "wk": _tile_rows(np.ascontiguousarray(Wk[:, sl])),
                "wv": _tile_rows(np.ascontiguousarray(Wv[:, sl])),
                "wo": _tile_rows(np.ascontiguousarray(Wo[sl, :])),
                "bq": np.ascontiguousarray((bq[sl] * scale).reshape(2, 128).T),
                "bk": np.ascontiguousarray(bk[sl].reshape(2, 128).T),
                "vb": vb,
                "onesd": np.ones((1, HD), _np_dt()),
                "vones": np.ones((128, ST, HPG, 1), _np_dt()),
            }
        )
    return in_maps


LAST_RESULT = None


def kernel(**inputs):
    global LAST_RESULT
    nc = get_nc()
    in_maps = shard_inputs(**inputs)
    trace = bool(int(os.environ.get("BASS_KERNEL_TRACE", "0")))
    res = run_bass_kernel_spmd(nc, in_maps, core_ids=list(range(8)), trace=trace)
    LAST_RESULT = res
    parts = [res.results[c]["out"] for c in range(8)]
    out = np.stack(
        [
            parts[0] + parts[1] + parts[2] + parts[3],
            parts[4] + parts[5] + parts[6] + parts[7],
        ]
    )
    return (out + inputs["bo"]).astype(np.float32)


# revision 15
# speedup vs baseline: 1.4153x; 1.3940x over previous
"""TRN2 Bass/Tile kernel: 16-head causal multi-head attention.

Problem: x[2,2048,1024], 16 heads x 64, causal softmax attention + out-proj.

Sharding (8 cores): core = b*4 + g  (b = batch 0..1, g = head-group 0..3).
Each core computes heads [4g, 4g+4) for batch b and the partial
out-projection  ctx_g @ Wo[g*256:(g+1)*256, :]  -> [2048, 1024].
Host sums the 4 partials per batch and adds bo.

On-device layout is fully "transposed" (feature-major):
  xT   [128, 8, 2048]  : xT[p, kt, s]  = x[b, s, kt*128+p]
  QT/KT[128, 2, 2048]  : QT[p, t, s]   = Q^T[t*128+p, s]   (d' = h*64+j on partitions)
  S^T  [128k, 512q]    : per (head, q-chunk, k-tile) block = K @ Q^T
  softmax: no max-subtraction (scores are O(1) by construction: exp is safe);
  denominators via a ones-column appended to V (row 64 of the ctx psum);
  ctx^T [128, 2, 2048] feeds the out-projection directly as lhsT.

Schedule (v4):
  - Head pairs (h0,h1)/(h2,h3) write one [128, 2, 512] two-bank S^T psum
    tile; their matmuls row-tile the PE array concurrently (K=64 halves)
    and ONE exp per pair halves ScalarE's per-op fixed cost.
  - 1/denom = exp(-ln(denom)) on ScalarE (Log+Exp share a table set) —
    the DVE InstReciprocal was 3-4us each and serialized every chunk tail.
  - Tails (normalize + out-projection) are emitted one chunk late,
    interleaved between projection chains, so nothing stalls the in-order
    PE queue; a dummy-matmul warmup burst keeps the PE HAM clock at 2.4GHz
    through the initial DMA.
  - PSUM: sT pairs 2x2 banks + ctx 4 banks = 8; proj/bcast/out-proj psums
    share the sT slots.
"""

import os
import sys

for _p in ("/opt/trn_rl_repo",):
    if _p not in sys.path:
        sys.path.insert(0, _p)

import numpy as np

import concourse.bass as bass
import concourse.mybir as mybir
import concourse.tile as tile
from concourse import bacc
from concourse.bass import ts
from concourse.bass_utils import run_bass_kernel_spmd

B, S, D, H, HD = 2, 2048, 1024, 16, 64
GROUPS, HPG, DG = 4, 4, 256  # head groups, heads/group, group width
KT = D // 128  # 8 k-tiles over D
ST = S // 128  # 16 s-tiles
CH = 512  # q-chunk width
QCH = S // CH  # 4 q-chunks
PIPE = 2  # ctx matmuls trail the S^T/exp stream by this many k-steps
F32 = mybir.dt.float32

_MM_DT_NAME = os.environ.get("BASS_MM_DT", "bf16")
MM_DT = {
    "f32r": mybir.dt.float32r,
    "f32": mybir.dt.float32,
    "bf16": mybir.dt.bfloat16,
}[_MM_DT_NAME]
RECIP_MODE = os.environ.get("BASS_RECIP", "act")  # act | plain
WARMUP = int(os.environ.get("BASS_WARMUP", "48"))


def _np_dt():
    import ml_dtypes

    return ml_dtypes.bfloat16 if _MM_DT_NAME == "bf16" else np.float32


def build_kernel_body(nc, tc, io):
    Exp = mybir.ActivationFunctionType.Exp
    Log = mybir.ActivationFunctionType.Ln

    consts = tc.alloc_tile_pool(name="consts", bufs=1)
    acts = tc.alloc_tile_pool(name="acts", bufs=1)
    work = tc.alloc_tile_pool(name="work", bufs=2)
    small = tc.alloc_tile_pool(name="small", bufs=2)
    psum = tc.alloc_tile_pool(name="psum", bufs=1, space="PSUM")

    # ---- constant loads (small + hot first) ------------------------------
    ones_sb = consts.tile([1, HD], MM_DT)
    nc.sync.dma_start(out=ones_sb, in_=io["onesd"])
    bq_sb = consts.tile([128, 2], F32)
    nc.sync.dma_start(out=bq_sb, in_=io["bq"])
    bk_sb = consts.tile([128, 2], F32)
    nc.sync.dma_start(out=bk_sb, in_=io["bk"])
    vb_sb = consts.tile([128, HPG, HD], F32)
    nc.sync.dma_start(out=vb_sb, in_=io["vb"])
    wq_sb = consts.tile([128, KT, DG], MM_DT)
    nc.sync.dma_start(out=wq_sb, in_=io["wq"])
    wk_sb = consts.tile([128, KT, DG], MM_DT)
    nc.sync.dma_start(out=wk_sb, in_=io["wk"])
    wv_sb = consts.tile([128, KT, DG], MM_DT)
    nc.sync.dma_start(out=wv_sb, in_=io["wv"])
    xt_sb = consts.tile([128, KT, S], MM_DT)
    for kt in range(KT):
        nc.sync.dma_start(out=xt_sb[:, kt, :], in_=io["xt"][:, kt, :])
    wo_sb = consts.tile([128, 2, 1024], MM_DT)
    nc.sync.dma_start(out=wo_sb, in_=io["wo"])

    # ---- persistent activations ----------------------------------------
    qt_sb = acts.tile([128, 2, S], MM_DT)  # Q^T (pre-scaled by 1/8 via host W/b)
    kt_sb = acts.tile([128, 2, S], MM_DT)  # K^T
    v_sb = acts.tile([128, ST, HPG, HD + 1], MM_DT)  # V blocks + ones column
    ctxT_sb = acts.tile([128, 2, S], MM_DT)  # normalized ctx^T
    nc.sync.dma_start(out=v_sb[:, :, :, HD : HD + 1], in_=io["vones"])

    # ---- PE warmup: keep HAM at full clock through the initial DMA ------
    for r in range(WARMUP):
        wu_ps = psum.tile([HD, HD], F32, tag="sT", bufs=2, name="wu_ps")
        nc.tensor.matmul(wu_ps, lhsT=ones_sb, rhs=ones_sb, start=True, stop=True)

    def proj_chains(c):
        """Q^T/K^T chunk c + V s-tiles of chunk c as a list of emit-thunks."""
        chains = []
        for t in range(2):
            for which, w_sb, b_sb, dst in (
                ("q", wq_sb, bq_sb, qt_sb),
                ("k", wk_sb, bk_sb, kt_sb),
            ):

                def chain(t=t, w_sb=w_sb, b_sb=b_sb, dst=dst, which=which):
                    ps = psum.tile([128, CH], F32, tag="sT", bufs=2, name=f"{which}_ps{c}{t}")
                    for kt in range(KT):
                        nc.tensor.matmul(
                            ps[:, 0:CH],
                            lhsT=w_sb[:, kt, ts(t, 128)],
                            rhs=xt_sb[:, kt, ts(c, CH)],
                            start=(kt == 0),
                            stop=(kt == KT - 1),
                        )
                    nc.vector.tensor_scalar_add(
                        out=dst[:, t, ts(c, CH)], in0=ps[:, 0:CH], scalar1=b_sb[:, t : t + 1]
                    )

                chains.append(chain)
        for st in range(4 * c, 4 * c + 4):

            def chain(st=st):
                ps = psum.tile([128, DG], F32, tag="sT", bufs=2, name=f"v_ps{st}")
                for kt in range(KT):
                    nc.tensor.matmul(
                        ps,
                        lhsT=xt_sb[:, kt, ts(st, 128)],
                        rhs=wv_sb[:, kt, :],
                        start=(kt == 0),
                        stop=(kt == KT - 1),
                    )
                nc.vector.tensor_add(
                    out=v_sb[:, st, :, 0:HD],
                    in0=ps.rearrange("p (h j) -> p h j", h=HPG),
                    in1=vb_sb,
                )

            chains.append(chain)
        return chains

    ctx_of = {}  # c -> list of 4 ctx psum tiles

    def emit_attn(c):
        """S^T/exp per head-pair, ctx matmuls trailing by PIPE k-steps."""
        nkt = (c + 1) * (CH // 128)
        ctx_ps = [
            psum.tile([HD + 1, CH], F32, tag="ctx", bufs=4, name=f"ctx_ps{c}_{h}")
            for h in range(HPG)
        ]
        ctx_of[c] = ctx_ps
        exps = [[None] * nkt for _ in range(2)]  # per pair

        def scores(i):
            off = max(0, 128 * i - CH * c)  # first unmasked column of this k-tile
            w = CH - off
            for pr in range(2):  # head pair (2*pr, 2*pr+1) -> tile t=pr
                sT_ps = psum.tile([128, 2, CH], F32, tag="sT", bufs=2, name="sT_ps")
                for sub in range(2):
                    pb = sub * 64
                    nc.tensor.matmul(
                        sT_ps[:, sub, off:CH],
                        lhsT=kt_sb[pb : pb + HD, pr, ts(i, 128)],
                        rhs=qt_sb[pb : pb + HD, pr, c * CH + off : (c + 1) * CH],
                        start=True,
                        stop=True,
                    )
                e = work.tile([128, 2, CH], MM_DT, tag="exp", bufs=8, name="e")
                nc.scalar.activation(
                    out=e[:, :, off:CH], in_=sT_ps[:, :, off:CH], func=Exp
                )
                if 128 * i + 128 > CH * c + off:  # crosses the diagonal: mask
                    nc.gpsimd.affine_select(
                        out=e[:, :, off:CH],
                        in_=e[:, :, off:CH],
                        pattern=[[0, 2], [1, w]],
                        base=off - (128 * i - CH * c),
                        channel_multiplier=-1,
                        compare_op=mybir.AluOpType.is_ge,
                        fill=0.0,
                    )
                exps[pr][i] = (e, off)

        def ctx(i):
            for h in range(HPG):
                e, off = exps[h // 2][i]
                nc.tensor.matmul(
                    ctx_of[c][h][:, off:CH],
                    lhsT=v_sb[:, i, h, :],
                    rhs=e[:, h % 2, off:CH],
                    start=(i == 0),
                    stop=(i == nkt - 1),
                )

        for i in range(nkt + PIPE):
            if i < nkt:
                scores(i)
            if i >= PIPE:
                ctx(i - PIPE)

    def tail_parts(c):
        """Normalize + out-projection thunks for chunk c (emitted late)."""
        parts = []
        for h in range(HPG):

            def norm(h=h, c=c):
                ctx_ps = ctx_of[c][h]
                t, pb = h // 2, (h % 2) * 64
                recip_mm = small.tile([1, CH], MM_DT, tag="recip", name="recip_mm")
                if RECIP_MODE == "act":
                    lg = small.tile([1, CH], F32, tag="lg", name="lg")
                    nc.scalar.activation(out=lg, in_=ctx_ps[HD : HD + 1, :], func=Log)
                    nc.scalar.activation(out=recip_mm, in_=lg, func=Exp, scale=-1.0)
                else:
                    nc.vector.reciprocal(out=recip_mm, in_=ctx_ps[HD : HD + 1, :])
                bc_ps = psum.tile([HD, CH], F32, tag="sT", bufs=2, name="bc_ps")
                nc.tensor.matmul(
                    bc_ps, lhsT=ones_sb, rhs=recip_mm, start=True, stop=True
                )
                bc_sb = small.tile([HD, CH], F32, tag="bc_sb", name="bc_sb")
                nc.vector.tensor_copy(out=bc_sb, in_=bc_ps)
                if pb == 0:
                    nc.vector.tensor_mul(
                        out=ctxT_sb[0:HD, t, ts(c, CH)], in0=ctx_ps[0:HD, :], in1=bc_sb
                    )
                else:
                    stg_sb = small.tile([HD, CH], MM_DT, tag="stg", name="stg_sb")
                    nc.vector.tensor_mul(out=stg_sb, in0=ctx_ps[0:HD, :], in1=bc_sb)
                    # DVE cannot shift partitions; bounce via SBUF->SBUF DMA
                    nc.sync.dma_start(
                        out=ctxT_sb[pb : pb + HD, t, ts(c, CH)], in_=stg_sb
                    )

            parts.append(norm)
        for st in range(4 * c, 4 * c + 4):

            def oproj(st=st):
                o_sb = work.tile([128, 1024], F32, tag="osb", bufs=3, name="o_sb")
                for nch in range(2):
                    o_ps = psum.tile([128, CH], F32, tag="sT", bufs=2, name="o_ps")
                    for t in range(2):
                        nc.tensor.matmul(
                            o_ps,
                            lhsT=ctxT_sb[:, t, ts(st, 128)],
                            rhs=wo_sb[:, t, ts(nch, CH)],
                            start=(t == 0),
                            stop=(t == 1),
                        )
                    nc.vector.tensor_copy(out=o_sb[:, ts(nch, CH)], in_=o_ps)
                nc.sync.dma_start(out=io["out"][ts(st, 128), :], in_=o_sb)

            parts.append(oproj)
        return parts

    def run_all(thunks):
        for th in thunks:
            th()

    def interleave(chains, parts):
        """chainA part0 chainB part1 ... — tails hide behind proj chains."""
        out = []
        n = max(len(chains), len(parts))
        for j in range(n):
            if j < len(chains):
                out.append(chains[j])
            if j < len(parts):
                out.append(parts[j])
        return out

    # ---- pipeline: tails run one chunk late ------------------------------
    run_all(proj_chains(0))
    emit_attn(0)
    run_all(interleave(proj_chains(1), tail_parts(0)))
    emit_attn(1)
    run_all(interleave(proj_chains(2), tail_parts(1)))
    emit_attn(2)
    run_all(interleave(proj_chains(3), tail_parts(2)))
    emit_attn(3)
    run_all(tail_parts(3))

    psum.release()
    small.release()
    work.release()
    acts.release()
    consts.release()


def build_nc():
    nc = bacc.Bacc("TRN2", target_bir_lowering=False, debug=False)
    io = {
        "xt": nc.dram_tensor("xt", [128, KT, S], MM_DT, kind="ExternalInput").ap(),
        "wq": nc.dram_tensor("wq", [128, KT, DG], MM_DT, kind="ExternalInput").ap(),
        "wk": nc.dram_tensor("wk", [128, KT, DG], MM_DT, kind="ExternalInput").ap(),
        "wv": nc.dram_tensor("wv", [128, KT, DG], MM_DT, kind="ExternalInput").ap(),
        "wo": nc.dram_tensor("wo", [128, 2, 1024], MM_DT, kind="ExternalInput").ap(),
        "bq": nc.dram_tensor("bq", [128, 2], F32, kind="ExternalInput").ap(),
        "bk": nc.dram_tensor("bk", [128, 2], F32, kind="ExternalInput").ap(),
        "vb": nc.dram_tensor("vb", [128, HPG, HD], F32, kind="ExternalInput").ap(),
        "onesd": nc.dram_tensor("onesd", [1, HD], MM_DT, kind="ExternalInput").ap(),
        "vones": nc.dram_tensor(
            "vones", [128, ST, HPG, 1], MM_DT, kind="ExternalInput"
        ).ap(),
        "out": nc.dram_tensor("out", [S, D], F32, kind="ExternalOutput").ap(),
    }
    with tile.TileContext(nc) as tc, nc.allow_low_precision(
        reason="reduced-precision matmul operand pipeline; accumulation stays fp32"
    ):
        build_kernel_body(nc, tc, io)
    nc.compile()
    return nc


_NC = None


def get_nc():
    global _NC
    if _NC is None:
        _NC = build_nc()
    return _NC


def _tile_rows(a, p=128):
    """[R, N] -> [128, R//128, N] with row r = kt*128 + p."""
    r, n = a.shape
    return np.ascontiguousarray(a.reshape(r // p, p, n).transpose(1, 0, 2)).astype(
        _np_dt()
    )


def shard_inputs(x, Wq, bq, Wk, bk, Wv, bv, Wo, bo):
    scale = 1.0 / np.sqrt(np.float32(HD))
    in_maps = []
    for core in range(8):
        b, g = divmod(core, GROUPS)
        sl = slice(g * DG, (g + 1) * DG)
        vb = np.ascontiguousarray(
            np.broadcast_to(bv[sl].reshape(HPG, HD)[None], (128, HPG, HD))
        ).astype(np.float32)
        in_maps.append(
            {
                "xt": _tile_rows(np.ascontiguousarray(x[b].T)),
                "wq": _tile_rows(np.ascontiguousarray(Wq[:, sl]) * scale),
                "wk": _tile_rows(np.ascontiguousarray(Wk[:, sl])),
                "wv": _tile_rows(np.ascontiguousarray(Wv[:, sl])),
                "wo": _tile_rows(np.ascontiguousarray(Wo[sl, :])),
                "bq": np.ascontiguousarray((bq[sl] * scale).reshape(2, 128).T),
                "bk": np.ascontiguousarray(bk[sl].reshape(2, 128).T),
                "vb": vb,
                "onesd": np.ones((1, HD), _np_dt()),
                "vones": np.ones((128, ST, HPG, 1), _np_dt()),
            }
        )
    return in_maps


LAST_RESULT = None


def kernel(**inputs):
    global LAST_RESULT
    inputs = {k: np.asarray(v) for k, v in inputs.items()}
    nc = get_nc()
    in_maps = shard_inputs(**inputs)
    trace = bool(int(os.environ.get("BASS_KERNEL_TRACE", "0")))
    res = run_bass_kernel_spmd(nc, in_maps, core_ids=list(range(8)), trace=trace)
    LAST_RESULT = res
    parts = [res.results[c]["out"] for c in range(8)]
    out = np.stack(
        [
            parts[0] + parts[1] + parts[2] + parts[3],
            parts[4] + parts[5] + parts[6] + parts[7],
        ]
    )
    return (out + inputs["bo"]).astype(np.float32)
